# revision 59
# baseline (speedup 1.0000x reference)
"""Trainium2 Bass kernel for nn_APM_9242769621730 (sparse temporal attention).

Data-parallel over (batch, time-half): 8 shards on 8 NeuronCores, no
collectives. Each core gets 10 time frames (8 own + 2 halo, zero-padded
at sequence boundaries) of one batch element and computes the 16
(t, neighbor) attention outputs for its 8 frames.

Precision: logits path (semantic conv, normalize, similarity) in float32r
(reduced fp32, full PE rate at moving dim >= 256). Attention probabilities
PT and values V in bf16 (quantized post-exp, ~0.4% each), accumulated in
fp32 PSUM.

Per (t, n) pair, with hw = 784 pixels (padded to 896 for the xbar
DMA-transpose), co = 32, c = 512:
  simT[qq, p] = sum_co k_norm[co, qq] * q_norm[co, p]        (TensorE, K=32)
  PT[qq, p]   = exp(TEMP * simT)  -> bf16                    (ScalarE)
  nnT[cc, p]  = sum_qq v[qq, cc] * PT[qq, p]     (bf16, full-width N=784)
  den[p]      = sum_qq PT[qq, p]                 (ones column pass)
  natt        = WnT.T @ nnT ; m = xatt * natt
  att[p]      = 1/(1+exp(-(WattT.T @ m)/den))
  out[cc, p]  = nnT * (att/den)   broadcast via GPSIMD partition_broadcast
"""

import numpy as np

import concourse.bacc as bacc
import concourse.tile as tile
from concourse import mybir
from concourse.bass_utils import run_bass_kernel_spmd

F32 = mybir.dt.float32
F32R = mybir.dt.float32r
BF16 = mybir.dt.bfloat16
AF = mybir.ActivationFunctionType

B, C, T, H, W = 4, 512, 16, 28, 28
HW = H * W          # 784
HWP = 896           # padded to 7*128 for xbar transpose
CO = 32
TH = T // 2         # 8 frames per core
NF = TH + 2         # frames incl halo
TEMP = 4.0
QP = 128            # hw partition-chunk size (last chunk has 16 valid rows)
NCHUNK = 7
HHW = HW // 2       # 392
NH = 2
CK = C // 128       # 4

_NC_CACHE = {}


def hs(h):
    return slice(h * HHW, (h + 1) * HHW)


def qcnt(q):
    return 16 if q == NCHUNK - 1 else QP


def _build():
    nc = bacc.Bacc("TRN2", target_bir_lowering=False, debug=False)
    xs_d = nc.declare_dram_parameter("xs", [C, NF, HW], F32, isOutput=False)
    wsx_d = nc.declare_dram_parameter("WsxT", [C, 2 * CO], F32, isOutput=False)
    wnt_d = nc.declare_dram_parameter("WnT", [C, CO], F32, isOutput=False)
    watt_d = nc.declare_dram_parameter("WattT", [CO, 1], F32, isOutput=False)
    out_d = nc.declare_dram_parameter("out", [C, TH, 2, HW], F32, isOutput=True)

    with tile.TileContext(nc) as tc:
        with (
            tc.tile_pool(name="const", bufs=1) as constp,
            tc.tile_pool(name="xraw", bufs=1) as xrawp,
            tc.tile_pool(name="xpool", bufs=1) as xpool,
            tc.tile_pool(name="xbfp", bufs=1) as xbfp,
            tc.tile_pool(name="xtpool", bufs=4) as xtpool,
            tc.tile_pool(name="snorm", bufs=1) as snormp,
            tc.tile_pool(name="ptp", bufs=3) as ptp,
            tc.tile_pool(name="nnp", bufs=2) as nnp,
            tc.tile_pool(name="bcp", bufs=2) as bcp,
            tc.tile_pool(name="rows", bufs=2) as rowsp,
            tc.tile_pool(name="semtmp", bufs=2) as semtmpp,
            tc.tile_pool(name="attm", bufs=2) as attmp,
            tc.tile_pool(name="ps_sim", bufs=2, space="PSUM") as ps_sim,
            tc.tile_pool(name="ps_big", bufs=2, space="PSUM") as ps_big,
            tc.tile_pool(name="ps_misc", bufs=2, space="PSUM") as ps_misc,
        ):
            # ---- constants / weights (rounded once) ----
            ones_f = constp.tile([128, 1], F32, tag="wstage", name="ones_f")
            nc.vector.memset(ones_f[:], 1.0)
            ones_col = constp.tile([128, 1], F32R, tag="ones_col")
            nc.vector.tensor_copy(out=ones_col[:], in_=ones_f[:])
            ones_bf = constp.tile([128, 1], BF16, tag="ones_bf")
            nc.vector.tensor_copy(out=ones_bf[:], in_=ones_f[:])
            onesr_f = constp.tile([1, 128], F32, tag="wstage2", name="onesr_f")
            nc.vector.memset(onesr_f[:], 1.0)
            ones_row = constp.tile([1, 128], F32R, tag="ones_row")
            nc.vector.tensor_copy(out=ones_row[:], in_=onesr_f[:])

            def load_weight(dram, shape, rearr, tag):
                wf = constp.tile(shape, F32, tag="wstage", name=tag + "_f")
                src = dram[:, :].rearrange(rearr, p=128) if rearr else dram[:, :]
                nc.sync.dma_start(out=wf[:], in_=src)
                wr = constp.tile(shape, F32R, tag=tag, name=tag)
                nc.scalar.copy(out=wr[:], in_=wf[:])
                return wr

            wsx = load_weight(wsx_d, [128, CK, 2 * CO], "(k p) m -> p k m", "wsx")
            wnt = load_weight(wnt_d, [128, CK, CO], "(k p) m -> p k m", "wnt")
            watt_f32r = load_weight(watt_d, [CO, 1], None, "watt")
            watt = constp.tile([CO, 1], BF16, tag="watt_bf")
            nc.vector.tensor_copy(out=watt[:], in_=watt_f32r[:])

            s_norm = snormp.tile([CO, NF, HW], F32R, tag="snorm")

            xt_tiles = {}
            xatt_tiles = {}

            def prep_frame(pf):
                """DMA frame pf; semantic+xatt+norm; bf16 transpose for V."""
                x_t = xrawp.tile([128, CK, HW], F32, tag="xraw")
                nc.sync.dma_start(
                    out=x_t[:],
                    in_=xs_d[:, pf, :].rearrange("(k p) f -> p k f", p=128),
                )
                x_r = xpool.tile([128, CK, HW], F32R, tag="x")
                nc.vector.tensor_copy(out=x_r[:], in_=x_t[:])

                s_sb = semtmpp.tile([CO, HW], F32, tag="semtmp")
                xatt_sb = attmp.tile([CO, HW], F32R, tag="xatt", bufs=4, name="xatt_sb")
                for h in range(NH):
                    ps = ps_misc.tile([2 * CO, HHW], F32, tag="misc")
                    for k in range(CK):
                        nc.tensor.matmul(
                            ps[:], wsx[:, k, :], x_r[:, k, hs(h)],
                            start=(k == 0), stop=(k == CK - 1),
                        )
                    nc.scalar.copy(out=s_sb[:, hs(h)], in_=ps[0:CO, :])
                    nc.scalar.copy(out=xatt_sb[:, hs(h)], in_=ps[CO:2 * CO, :])
                xatt_tiles[pf] = xatt_sb

                s2 = semtmpp.tile([CO, HW], F32R, tag="semtmp")
                nc.vector.tensor_mul(s2[:], s_sb[:], s_sb[:])

                # rsqrt rows via ln+exp on ACT
                r_a = rowsp.tile([1, HW], F32, tag="frow", bufs=3, name="r_a")
                for h in range(NH):
                    ps = ps_misc.tile([2 * CO, HHW], F32, tag="misc")
                    nc.tensor.matmul(
                        ps[0:1, :], ones_col[0:CO, :], s2[:, hs(h)],
                        start=True, stop=True,
                    )
                    nc.scalar.activation(out=r_a[:, hs(h)], in_=ps[0:1, :], func=AF.Ln)
                r_b = rowsp.tile([1, HW], F32, tag="frow", bufs=3, name="r_b")
                nc.scalar.activation(out=r_b[:], in_=r_a[:], func=AF.Exp, scale=-0.5)
                r_c = rowsp.tile([1, HW], F32R, tag="frow", bufs=3, name="r_c")
                with nc.allow_low_precision(reason="f32r rounding of rsqrt row"):
                    nc.vector.tensor_scalar_min(r_c[:], r_b[:], 1e12)

                for h in range(NH):
                    ps = ps_misc.tile([2 * CO, HHW], F32, tag="misc")
                    nc.tensor.matmul(
                        ps[:CO, :], ones_row[0:1, 0:CO], r_c[:, hs(h)],
                        start=True, stop=True,
                    )
                    nc.vector.tensor_mul(
                        s_norm[:, pf, hs(h)], s_sb[:, hs(h)], ps[:CO, :]
                    )

                # bf16 downcast + zero pad tail, then xbar DMA transpose
                x_bf = xbfp.tile([128, CK, HWP], BF16, tag="xbf")
                nc.gpsimd.tensor_copy(out=x_bf[:, :, 0:HW], in_=x_t[:])
                nc.gpsimd.memset(x_bf[:, :, HW:HWP], 0.0)
                xt_t = xtpool.tile([QP, NCHUNK, C], BF16, tag="xt")
                for k in range(CK):
                    nc.sync.dma_start_transpose(
                        out=xt_t[:, :, k * 128:(k + 1) * 128], in_=x_bf[:, k, :]
                    )
                xt_tiles[pf] = xt_t

            def produce_pt(kf, qf):
                """simT (f32r) + exp -> PT (bf16), padded for transposability."""
                pt = ptp.tile([QP, NCHUNK, HWP], BF16, tag="pt", name="pt")
                # zero the p-pad columns and the 112 dead rows of the
                # last qq chunk (exp overwrites its 16 valid rows)
                nc.gpsimd.memset(pt[:, :, HW:HWP], 0.0)
                nc.gpsimd.memset(pt[:, NCHUNK - 1, 0:HW], 0.0)
                for q in range(NCHUNK):
                    cnt = qcnt(q)
                    for h in range(NH):
                        ps = ps_sim.tile([QP, HHW], F32, tag="sim")
                        nc.tensor.matmul(
                            ps[0:cnt, :],
                            s_norm[:, kf, q * QP:q * QP + cnt],
                            s_norm[:, qf, hs(h)],
                            start=True, stop=True,
                        )
                        nc.scalar.activation(
                            out=pt[0:cnt, q, hs(h)], in_=ps[0:cnt, :],
                            func=AF.Exp, scale=TEMP,
                        )
                return pt

            def transpose_pt(pt):
                """P(k->q)^T = P(q->k) via xbar DMA transpose."""
                pt_next = ptp.tile([QP, NCHUNK, HWP], BF16, tag="pt", name="pt_next")
                for q_in in range(NCHUNK):
                    nc.sync.dma_start_transpose(
                        out=pt_next[:, :, q_in * QP:(q_in + 1) * QP],
                        in_=pt[:, q_in, :],
                    )
                return pt_next

            def do_pair(i, n, kf, qf, xatt_sb, pt):
                """nn/att/output phases for a pair with PT already made."""
                xt_t = xt_tiles[kf]

                # --- nn matmul (bf16, full-width) + denominator ---
                nn_sb = nnp.tile([128, CK, HW], F32R, tag="nn")
                den = rowsp.tile([1, HW], F32, tag="prow", bufs=5, name="den")
                for cc in range(CK):
                    psn = ps_big.tile([128, HW], F32, tag="big", name="ps_nn")
                    for q in range(NCHUNK):
                        for lo, hi in ((0, 512), (512, HW)):
                            nc.tensor.matmul(
                                psn[:, lo:hi],
                                xt_t[:, q, cc * 128:(cc + 1) * 128],
                                pt[:, q, lo:hi],
                                start=(q == 0), stop=(q == NCHUNK - 1),
                            )
                    nc.scalar.copy(out=nn_sb[:, cc, :], in_=psn[:])
                psd = ps_big.tile([128, HW], F32, tag="big", name="ps_den")
                for q in range(NCHUNK):
                    for lo, hi in ((0, 512), (512, HW)):
                        nc.tensor.matmul(
                            psd[0:1, lo:hi], ones_bf[:], pt[:, q, lo:hi],
                            start=(q == 0), stop=(q == NCHUNK - 1),
                        )
                nc.vector.tensor_copy(out=den[:], in_=psd[0:1, :])

                # --- natt = WnT.T @ nnT ; m = xatt * natt ---
                m_sb = attmp.tile([CO, HW], BF16, tag="m", bufs=1, name="m_sb")
                for h in range(NH):
                    psm = ps_misc.tile([2 * CO, HHW], F32, tag="misc")
                    for k in range(CK):
                        nc.tensor.matmul(
                            psm[0:CO, :], wnt[:, k, :], nn_sb[:, k, hs(h)],
                            start=(k == 0), stop=(k == CK - 1),
                        )
                    nc.vector.tensor_mul(
                        m_sb[:, hs(h)], psm[0:CO, :], xatt_sb[:, hs(h)]
                    )

                # --- att row + scale row ---
                rden = rowsp.tile([1, HW], F32, tag="prow", bufs=5, name="rden")
                nc.vector.reciprocal(out=rden[:], in_=den[:])
                att_a = rowsp.tile([1, HW], F32, tag="prow", bufs=5, name="att_a")
                for h in range(NH):
                    psa = ps_misc.tile([2 * CO, HHW], F32, tag="misc")
                    nc.tensor.matmul(
                        psa[0:1, :], watt[:], m_sb[:, hs(h)], start=True, stop=True
                    )
                    nc.vector.tensor_mul(att_a[:, hs(h)], psa[0:1, :], rden[:, hs(h)])
                # srow = att * rden = rden / (1 + exp(-att_in))
                e_row = rowsp.tile([1, HW], F32, tag="prow", bufs=5, name="e_row")
                nc.scalar.activation(out=e_row[:], in_=att_a[:], func=AF.Exp, scale=-1.0)
                nc.vector.tensor_scalar_add(e_row[:], e_row[:], 1.0)
                att_r = rowsp.tile([1, HW], F32, tag="prow", bufs=5, name="att_r")
                nc.vector.reciprocal(out=att_r[:], in_=e_row[:])
                srow = rowsp.tile([1, HW], BF16, tag="prow", bufs=5, name="srow")
                nc.vector.tensor_mul(srow[:], att_r[:], rden[:])

                # --- broadcast scale (GPSIMD), apply, DMA out ---
                sb_bc = bcp.tile([128, HW], BF16, tag="bc")
                nc.gpsimd.partition_broadcast(sb_bc[:], srow[:], channels=128)
                out_v = out_d[:, i, n, :].rearrange("(k p) f -> p k f", p=128)
                for cc in range(CK):
                    nc.vector.tensor_mul(nn_sb[:, cc, :], nn_sb[:, cc, :], sb_bc[:])
                    nc.sync.dma_start(
                        out=out_v[:, cc, :], in_=nn_sb[:, cc, :].bitcast(F32)
                    )

            # ---- main schedule: preps batched two frames at a time so the
            # rsqrt ACT-table switches cluster once per block ----
            prep_frame(0)
            prep_frame(1)
            next_pt = None
            for it in range(0, TH, 2):
                prep_frame(it + 2)
                prep_frame(it + 3)
                for i in (it, it + 1):
                    qf = i + 1
                    pt_a = next_pt if next_pt is not None else produce_pt(i, qf)
                    pt_b = produce_pt(i + 2, qf)
                    next_pt = transpose_pt(pt_b) if i < TH - 1 else None
                    do_pair(i, 0, i, qf, xatt_tiles[qf], pt_a)
                    do_pair(i, 1, i + 2, qf, xatt_tiles[qf], pt_b)
                    del xt_tiles[i], xatt_tiles[qf]

    nc.compile()
    return nc


def _get_nc():
    if "nc" not in _NC_CACHE:
        _NC_CACHE["nc"] = _build()
    return _NC_CACHE["nc"]


def kernel(x, Ws, Wx, Wn, Watt):
    x = np.asarray(x, dtype=np.float32)
    nc = _get_nc()

    xr = x.reshape(B, C, T, HW)
    xpad = np.zeros((B, C, T + 2, HW), dtype=np.float32)
    xpad[:, :, 1 : T + 1] = xr

    wsx = np.ascontiguousarray(
        np.concatenate([np.asarray(Ws, np.float32), np.asarray(Wx, np.float32)], 0).T
    )  # [C, 2*CO]
    wnt = np.ascontiguousarray(np.asarray(Wn, np.float32).T)
    watt = np.ascontiguousarray(np.asarray(Watt, np.float32).T)   # [CO, 1]

    in_maps = []
    shard_meta = []
    for core in range(8):
        bi, half = divmod(core, 2)
        t0 = half * TH
        xs = np.ascontiguousarray(xpad[bi, :, t0 : t0 + NF])      # [C, NF, HW]
        in_maps.append({"xs": xs, "WsxT": wsx, "WnT": wnt, "WattT": watt})
        shard_meta.append((bi, t0))

    res = run_bass_kernel_spmd(nc, in_maps, core_ids=list(range(8)), trace=False)

    out = np.empty((B, C, 3 * T, H, W), dtype=np.float32)
    out[:, :, 1::3] = x
    for core in range(8):
        bi, t0 = shard_meta[core]
        nbr = res.results[core]["out"].reshape(C, TH, 2, H, W)
        out[bi, :, 3 * t0 : 3 * (t0 + TH) : 3] = nbr[:, :, 0]
        out[bi, :, 3 * t0 + 2 : 3 * (t0 + TH) + 2 : 3] = nbr[:, :, 1]
    # boundary neighbor slots are exactly zero (attention over a zero frame)
    out[:, :, 0] = 0.0
    out[:, :, 3 * T - 1] = 0.0
    return out
